# revision 27
# baseline (speedup 1.0000x reference)
"""Trainium2 Bass kernel for nn_CCNN (banded continuous-kernel conv).

Math: the reference builds a full (B,L,L) pairwise tensor, runs a tiny
scalar->8x8-matrix MLP on every (i,j) pair, masks to the band
j in [i-5, i-1], and contracts:  x_new[b,i,:] = x[b,i,:] @ sum_j kv[b,i,j].
Only the 5 sub-diagonals survive the band mask, so we evaluate the MLP
only on the 5 offsets o=1..5 per row:  dt_o = t_i - t_{i-o}.

Layout on device (per core, R=256 rows of the flattened (B*L) row axis):
  - hidden dims on partitions, rows on the free dim (256 columns); the
    5 offsets are block-diagonal in the partition dim, split 3 (A-half,
    base partition 0) + 2 (B-half, base 64: the PE quadrant rule needs
    lhsT/rhs base in {0,32,64} and equal, and 5*32=160 > 128 at h2).
  - every bias is folded into the preceding matmul through a constant-1
    row that each matmul regenerates for the next one (lhsT gets an
    extra ones-generating column), so all relu stages are bias-free and
    the bias pack / per-stage bias reads of the old design are gone.
  - dt is computed on the host (it is just 5 shifted subtractions) and
    shipped as the 6-row rhs [dt_1..dt_5; ones] of the first matmul.
  - the band mask is NOT applied on device: rows i >= 5 have all 5
    offsets valid, so only the 20 rows with i < 5 (5 per batch) are
    wrong; the host overwrites those with an exact numpy computation.
    The constant +5*B4 term rides the ones row into the W4 matmul.
  - x0 = emb[features] is gathered on the host and shipped broadcast to
    the 64 (c,d) partitions (xe); the per-row x contraction is
    elementwise-multiply + selection-matmul as before.
  - matmuls run in fp32r; weights are pre-rounded on the host,
    activations are rounded by their producing instruction writing an
    fp32r tile.  A warm-up matmul at t~0.8us starts the PE p-state ramp
    early so real matmuls run at mid/full speed.
  - DMA plan (HWDGE dispatches serialize at 625ns each, and every DMA
    completion costs 900ns semaphore propagation, so order matters):
    SP/HWDGE carries [dt+W1 hot pack] then [layer-0 weights in 2
    pieces]; the free Pool/SWDGE queue carries [layer-1 weights] and
    [xe+selection pack] in parallel.
"""

import numpy as np

F = 2
KW = 5  # band width (kernel size)
CIN = 8
COUT = 8
H1, H2, H3 = 16, 32, 16
B, L = 4, 512
NCORES = 8
R = (B * L) // NCORES  # 256 rows per core

# offsets 0..2 are the A-half (base partition 0), 3..4 the B-half (base 64)
OA, OB = 3, 2

TRACE = False
LAST_RESULTS = None
F32R_ENABLED = True  # fp32r (TF32-like) matmuls; flip False for full fp32

_cache = {}

# ---- pack column layouts -------------------------------------------------
# d1 pack (6, 450): [dt6 (6,256) | W1pack0 (6,97) | W1pack1 (6,97)]
D1_W = R + 2 * 97  # 450
# per-layer weight pack (97, 371):
#   W2A (49p,  0: 97)   rows 0:48 blkdiag(W2 x3), row 48 = [B2 x3 | 1@96]
#   W2B (33p@64, 97:162) rows 64:96 blkdiag(W2 x2), row 96 = [B2 x2 | 1@64]
#   W3A (97p, 162:211)  rows 0:96 blkdiag(W3 x3), row 96 = [B3 x3 | 1@48]
#   W3B (65p, 211:243)  rows 0:64 blkdiag(W3 x2), row 64 = [B3 x2]
#   W4A (49p, 243:307)  rows 0:48 tile(W4, 3),   row 48 = 5*B4
#   W4B (32p, 307:371)  rows 0:32 tile(W4, 2)
WL_W = 97 + 65 + 49 + 32 + 64 + 64  # 371
C_W2A, C_W2B, C_W3A, C_W3B, C_W4A, C_W4B = 0, 97, 162, 211, 243, 307
# tail pack (64, 328): [xe (64,256) | SelX (64,64) | sel8 (64,8)]
TAIL_W = R + 64 + 8


def _round_f32r(x):
    """Round-to-nearest keeping 11 mantissa bits (hardware fp32r format)."""
    if not F32R_ENABLED:
        return np.ascontiguousarray(x, np.float32)
    b = np.ascontiguousarray(x, np.float32).view(np.uint32)
    b = (b + np.uint32(0x800)) & np.uint32(0xFFFFF000)
    return b.view(np.float32)


def _blkdiag(w, n):
    p, q = w.shape
    out = np.zeros((n * p, n * q), np.float32)
    for o in range(n):
        out[o * p : (o + 1) * p, o * q : (o + 1) * q] = w
    return out


def _build_pack_arrays(emb, W1, B1, W2, B2, W3, B3, W4, B4):
    """Host-side constant packs (everything except dt6/xe, which are
    per-core).  Returns (w1part (6,194), wl[2] (97,371), selpart (64,72))."""
    w1part = np.zeros((6, 2 * 97), np.float32)
    wls = []
    for f in range(F):
        w1f = W1[f].reshape(H1).astype(np.float32)
        # W1pack (6, 97): rows 0:5 multiply dt_1..dt_5, row 5 multiplies 1.0
        w1p = np.zeros((6, 97), np.float32)
        for o in range(OA):
            w1p[o, o * H1 : (o + 1) * H1] = w1f
        for o in range(OB):
            w1p[OA + o, 64 + o * H1 : 64 + (o + 1) * H1] = w1f
        w1p[5, 0:48] = np.tile(B1[f], OA)
        w1p[5, 48] = 1.0  # ones row for mm2A (h1 partition 48)
        w1p[5, 64:96] = np.tile(B1[f], OB)
        w1p[5, 96] = 1.0  # ones row for mm2B (h1 partition 96)
        w1part[:, f * 97 : (f + 1) * 97] = w1p

        wl = np.zeros((97, WL_W), np.float32)
        # W2A (49, 97)
        wl[0:48, C_W2A : C_W2A + 96] = _blkdiag(W2[f], OA)
        wl[48, C_W2A : C_W2A + 96] = np.tile(B2[f], OA)
        wl[48, C_W2A + 96] = 1.0  # h2A ones row (partition 96)
        # W2B (33, 65) at base partition 64
        wl[64:96, C_W2B : C_W2B + 64] = _blkdiag(W2[f], OB)
        wl[96, C_W2B : C_W2B + 64] = np.tile(B2[f], OB)
        wl[96, C_W2B + 64] = 1.0  # h2B ones row (partition 64)
        # W3A (97, 49)
        wl[0:96, C_W3A : C_W3A + 48] = _blkdiag(W3[f], OA)
        wl[96, C_W3A : C_W3A + 48] = np.tile(B3[f], OA)
        wl[96, C_W3A + 48] = 1.0  # h3A ones row (partition 48)
        # W3B (65, 32)
        wl[0:64, C_W3B : C_W3B + 32] = _blkdiag(W3[f], OB)
        wl[64, C_W3B : C_W3B + 32] = np.tile(B3[f], OB)
        # W4A (49, 64)
        wl[0:48, C_W4A : C_W4A + 64] = np.tile(W4[f], (OA, 1))
        wl[48, C_W4A : C_W4A + 64] = KW * B4[f]  # nmask=5 for i>=5 rows
        # W4B (32, 64)
        wl[0:32, C_W4B : C_W4B + 64] = np.tile(W4[f], (OB, 1))
        wls.append(_round_f32r(wl))

    selx = np.zeros((CIN * COUT, CIN * COUT), np.float32)
    for cp in range(CIN):
        for dp in range(COUT):
            for d in range(COUT):
                selx[cp * COUT + dp, dp * COUT + d] = 1.0
    sel8 = np.tile(np.eye(COUT, dtype=np.float32), (CIN, 1))
    selpart = _round_f32r(np.concatenate([selx, sel8], axis=1))  # (64, 72)
    return _round_f32r(w1part), wls, selpart


def _build_nc():
    import concourse.bacc as bacc
    import concourse.bass as cbass
    import concourse.mybir as mybir
    from concourse.tile import TileContext

    F32 = mybir.dt.float32
    F32R = mybir.dt.float32r if F32R_ENABLED else mybir.dt.float32
    RELU = mybir.ActivationFunctionType.Relu

    # Route the Bass-preamble const-AP memsets (4 ops, pre-barrier) to
    # DVE: on Pool they serialize at 95ns each and push the entry barrier
    # out; DVE runs them in ~65ns each.
    _orig_memset = cbass.BassGpSimd.memset
    cbass.BassGpSimd.memset = lambda self, ap, c: self.bass.vector.memset(ap, c)
    # Dispatch the hot-pack DMA BEFORE the entry barrier (SP queue is idle
    # from t~25): its ~2.2us dispatch+transfer+sem-prop latency then
    # overlaps the barrier instead of following it.  Sync is manual: the
    # DMA bumps a semaphore that the first matmul waits on.
    _orig_barrier = cbass.Bass.all_engine_barrier
    def _barrier_hook(self, *a, **k):
        if not hasattr(self, "_early_dma"):
            dram = self.dram_tensor("d1", (6, D1_W), F32R, kind="ExternalInput")
            sb = self.alloc_sbuf_tensor("d1t_early", [6, D1_W], F32R)
            sem = self.alloc_semaphore("d1_early_sem")
            ins = self.sync.dma_start(out=sb.ap(), in_=dram.ap())
            ins.then_inc(sem, 16)
            self._early_dma = (dram, sb, sem)
        return _orig_barrier(self, *a, **k)
    cbass.Bass.all_engine_barrier = _barrier_hook
    try:
        nc = bacc.Bacc("TRN2", debug=False)
    finally:
        cbass.BassGpSimd.memset = _orig_memset
        cbass.Bass.all_engine_barrier = _orig_barrier
    d1_d, d1t_early, d1_sem = nc._early_dma
    wl0_d = nc.dram_tensor("wl0", (97, WL_W), F32R, kind="ExternalInput")
    wl1_d = nc.dram_tensor("wl1", (97, WL_W), F32R, kind="ExternalInput")
    tail_d = nc.dram_tensor("tailp", (64, TAIL_W), F32R, kind="ExternalInput")
    out_d = nc.dram_tensor("out", (CIN * COUT, R), F32R, kind="ExternalOutput")

    with TileContext(nc) as tc:
        with (
            tc.tile_pool(name="const", bufs=1) as cpool,
            tc.tile_pool(name="work", bufs=2) as wpool,
            tc.tile_pool(name="psum", bufs=2, space="PSUM") as ppool,
        ):
            # ---- warm-ups (no DMA deps, run during the DMA phase) ----
            # ACT: the dummy relu pulls the 1.3us LoadActFuncSet early.
            warm = cpool.tile([1, 1], F32, tag="warm")
            nc.vector.memset(warm, 0.0)
            nc.scalar.activation(out=warm, in_=warm, func=RELU)
            # PE: dummy matmuls start the p-state ramp clock so the real
            # matmuls run at mid/full speed instead of cold.  The first one
            # reads the Bass preamble const tensor (written before the
            # entry barrier), so it issues with no DMA/memset dependency.
            import concourse.mybir as _mybir
            const1 = nc.const_aps.aps[(_mybir.dt.float32, 1.0)]
            wps = ppool.tile([1, 1], F32, tag="msum", bufs=2, name="warmps")
            nc.tensor.matmul(wps, const1[0:1, 0:1], const1[0:1, 0:1], start=True, stop=True)
            wmm = cpool.tile([1, 4], F32, tag="wmm")
            nc.vector.memset(wmm, 0.25)
            wps2 = ppool.tile([1, 4], F32, tag="msum", bufs=2, name="warmps2")
            nc.tensor.matmul(wps2, wmm[0:1, 0:1], wmm[0:1, 0:4], start=True, stop=True)

            # ---- DMAs ----
            # SP/HWDGE: layer-0 weights split so the mm2 piece lands
            # before act1 completes (the hot pack went pre-barrier).
            d1t = d1t_early
            wl0t = cpool.tile([97, WL_W], F32R, tag="wl0")
            nc.sync.dma_start(out=wl0t[:, 0:C_W3A], in_=wl0_d.ap()[:, 0:C_W3A])
            nc.sync.dma_start(out=wl0t[:, C_W3A:WL_W], in_=wl0_d.ap()[:, C_W3A:WL_W])
            # Pool/SWDGE (otherwise idle): layer-1 W2 piece + tail pack;
            # layer-1's second piece rides HWDGE slot 3 so its transfer
            # is not queued behind the tail pack on the DMA engines.
            wl1t = cpool.tile([97, WL_W], F32R, tag="wl1")
            nc.gpsimd.dma_start(out=wl1t[:, 0:C_W3A], in_=wl1_d.ap()[:, 0:C_W3A])
            nc.sync.dma_start(out=wl1t[:, C_W3A:C_W4A], in_=wl1_d.ap()[:, C_W3A:C_W4A])
            nc.sync.dma_start(out=wl1t[:, C_W4A:WL_W], in_=wl1_d.ap()[:, C_W4A:WL_W])
            tailt = cpool.tile([64, TAIL_W], F32R, tag="tail")
            nc.gpsimd.dma_start(out=tailt, in_=tail_d.ap())
            wlt = [wl0t, wl1t]

            d1ap = d1t.ap()
            dt6 = d1ap[0:6, 0:R]
            def w1s(f):
                return d1ap[0:6, R + f * 97 : R + (f + 1) * 97]

            xe = tailt[0:64, 0:R]
            selx = tailt[0:64, R : R + 64]
            sel8 = tailt[0:64, R + 64 : R + 72]

            # ---- MLP: both layers interleaved stage by stage ----
            h1ps, h1 = {}, {}
            h2Aps, h2Bps, h2A, h2B = {}, {}, {}, {}
            h3Aps, h3Bps, h3A, h3B = {}, {}, {}, {}
            msum = {}

            mm1_instrs = []
            for f in range(F):
                h1ps[f] = ppool.tile([97, R], F32, tag="mm", bufs=5, name=f"h1ps{f}")
                mm1_instrs.append(
                    nc.tensor.matmul(h1ps[f], w1s(f), dt6, start=True, stop=True)
                )
            h1[0] = wpool.tile([97, R], F32R, tag="h1", name="h1_0")
            nc.vector.tensor_relu(h1[0], h1ps[0])
            h1[1] = wpool.tile([97, R], F32R, tag="h1", name="h1_1")
            nc.scalar.activation(out=h1[1], in_=h1ps[1], func=RELU)

            for f in range(F):
                h2Aps[f] = ppool.tile([97, R], F32, tag="mm", bufs=5, name=f"h2Aps{f}")
                nc.tensor.matmul(
                    h2Aps[f], wlt[f][0:49, C_W2A : C_W2A + 97], h1[f][0:49, :],
                    start=True, stop=True,
                )
                h2Bps[f] = ppool.tile([65, R], F32, tag="mm", bufs=5, name=f"h2Bps{f}")
                nc.tensor.matmul(
                    h2Bps[f], wlt[f][64:97, C_W2B : C_W2B + 65], h1[f][64:97, :],
                    start=True, stop=True,
                )
            h2A[0] = wpool.tile([97, R], F32R, tag="h2A", name="h2A_0")
            nc.vector.tensor_relu(h2A[0], h2Aps[0])
            h2B[0] = wpool.tile([65, R], F32R, tag="h2B", name="h2B_0")
            nc.scalar.activation(out=h2B[0], in_=h2Bps[0], func=RELU)
            h2A[1] = wpool.tile([97, R], F32R, tag="h2A", name="h2A_1")
            nc.vector.tensor_relu(h2A[1], h2Aps[1])
            h2B[1] = wpool.tile([65, R], F32R, tag="h2B", name="h2B_1")
            nc.scalar.activation(out=h2B[1], in_=h2Bps[1], func=RELU)

            # Layer 0's h3 stage and W4 matmuls are emitted BEFORE layer
            # 1's h3 matmuls: the PE queue is in-order, and layer 1's mm3
            # can be gated on its (late) weight DMA — msum0 must not queue
            # behind that.
            def mm3(f):
                h3Aps[f] = ppool.tile([49, R], F32, tag="mm", bufs=5, name=f"h3Aps{f}")
                nc.tensor.matmul(
                    h3Aps[f], wlt[f][0:97, C_W3A : C_W3A + 49], h2A[f][0:97, :],
                    start=True, stop=True,
                )
                h3Bps[f] = ppool.tile([32, R], F32, tag="mm", bufs=5, name=f"h3Bps{f}")
                nc.tensor.matmul(
                    h3Bps[f], wlt[f][0:65, C_W3B : C_W3B + 32], h2B[f][0:65, :],
                    start=True, stop=True,
                )

            def mm4(f):
                msum[f] = ppool.tile([64, R], F32, tag="msum", bufs=2, name=f"msum{f}")
                nc.tensor.matmul(
                    msum[f], wlt[f][0:49, C_W4A : C_W4A + 64], h3A[f][0:49, :],
                    start=True, stop=False,
                )
                nc.tensor.matmul(
                    msum[f], wlt[f][0:32, C_W4B : C_W4B + 64], h3B[f][0:32, :],
                    start=False, stop=True,
                )

            mm3(0)
            h3A[0] = wpool.tile([49, R], F32R, tag="h3A", name="h3A_0")
            nc.vector.tensor_relu(h3A[0], h3Aps[0])
            h3B[0] = wpool.tile([32, R], F32R, tag="h3B", name="h3B_0")
            nc.scalar.activation(out=h3B[0], in_=h3Bps[0], func=RELU)
            mm4(0)
            mm3(1)
            h3A[1] = wpool.tile([49, R], F32R, tag="h3A", name="h3A_1")
            nc.vector.tensor_relu(h3A[1], h3Aps[1])
            h3B[1] = wpool.tile([32, R], F32R, tag="h3B", name="h3B_1")
            nc.scalar.activation(out=h3B[1], in_=h3Bps[1], func=RELU)
            mm4(1)

            # ---- serial x-contraction tail ----
            prod0 = wpool.tile([64, R], F32R, tag="prod")
            nc.vector.tensor_mul(out=prod0, in0=msum[0], in1=xe)
            selxps = ppool.tile([64, R], F32, tag="tailps", bufs=1, name="selxps")
            nc.tensor.matmul(selxps, selx, prod0, start=True, stop=True)
            # msum1 moves PSUM->SBUF on DVE right after prod0 (the copy
            # must exist: two PSUM operands are illegal for DVE tensor
            # ops, and Pool cannot read PSUM at all)
            msum1s = wpool.tile([64, R], F32, tag="msum1s")
            nc.scalar.copy(out=msum1s, in_=msum[1])
            # prod1 (SBUF) is DMA'd out directly; the final 8-way c-sum
            # out[d] = sum_c prod1[(c,d)] happens during the host gather
            prod1 = wpool.tile([64, R], F32R, tag="prod")
            nc.vector.tensor_mul(out=prod1, in0=selxps, in1=msum1s)
            nc.sync.dma_start(out=out_d.ap(), in_=prod1)

    # The early-DMA wait is attached after TileContext scheduling (the
    # scheduler's sim cannot see the pre-barrier DMA and would deadlock
    # on an in-block wait instruction).
    for ins in mm1_instrs:
        ins.wait_op(d1_sem, 16, "sem-ge")
    nc.finalize()
    return nc


def _per_core_inputs(times, features, emb, core):
    rows = np.arange(core * R, (core + 1) * R)
    b = rows // L
    i = rows % L

    dt6 = np.zeros((6, R), np.float32)
    tcur = times[b, i]
    for o in range(1, KW + 1):
        valid = i >= o
        dt6[o - 1, valid] = tcur[valid] - times[b[valid], i[valid] - o]
    dt6[5, :] = 1.0

    x0 = emb[features[b, i].astype(np.int64)].astype(np.float32)  # (R, 8)
    xe = np.repeat(np.ascontiguousarray(x0.T), COUT, axis=0)  # (64, R), c-major
    return _round_f32r(dt6), xe


def _fixup_head(out, times, features, emb, W1, B1, W2, B2, W3, B3, W4, B4):
    """Rows i < 5 have fewer than 5 valid band offsets; the device assumes
    all 5 (dt=0, nmask=5), so overwrite them with the exact computation."""
    for b in range(B):
        x = emb[features[b, :KW].astype(np.int64)].astype(np.float32)  # (5, 8)
        for f in range(F):
            xn = np.zeros((KW, CIN), np.float32)
            for i in range(KW):
                K = np.zeros((CIN, COUT), np.float32)
                for o in range(1, i + 1):
                    s = np.float32(times[b, i] - times[b, i - o])
                    h = np.maximum(s * W1[f].reshape(H1) + B1[f], 0.0)
                    h = np.maximum(h @ W2[f] + B2[f], 0.0)
                    h = np.maximum(h @ W3[f] + B3[f], 0.0)
                    K += (h @ W4[f] + B4[f]).reshape(CIN, COUT)
                xn[i] = x[i] @ K
            x = xn
        out[b, :KW, :] = x
    return out


def kernel(times, features, emb, W1, B1, W2, B2, W3, B3, W4, B4):
    global LAST_RESULTS
    from concourse.bass_utils import run_bass_kernel_spmd

    times = np.asarray(times, dtype=np.float32)
    features = np.asarray(features)
    emb = np.asarray(emb, dtype=np.float32)
    W1, B1 = np.asarray(W1, np.float32), np.asarray(B1, np.float32)
    W2, B2 = np.asarray(W2, np.float32), np.asarray(B2, np.float32)
    W3, B3 = np.asarray(W3, np.float32), np.asarray(B3, np.float32)
    W4, B4 = np.asarray(W4, np.float32), np.asarray(B4, np.float32)

    if "nc" not in _cache:
        _cache["nc"] = _build_nc()
    nc = _cache["nc"]

    w1part, wls, selpart = _build_pack_arrays(emb, W1, B1, W2, B2, W3, B3, W4, B4)

    in_maps = []
    for core in range(NCORES):
        dt6, xe = _per_core_inputs(times, features, emb, core)
        d1 = np.zeros((6, D1_W), np.float32)
        d1[:, 0:R] = dt6
        d1[:, R:] = w1part
        tailp = np.concatenate([xe, selpart], axis=1).astype(np.float32)
        in_maps.append({"d1": d1, "wl0": wls[0], "wl1": wls[1], "tailp": tailp})

    res = run_bass_kernel_spmd(nc, in_maps, list(range(NCORES)), trace=TRACE)
    LAST_RESULTS = res

    out = np.zeros((B * L, CIN), np.float32)
    for core in range(NCORES):
        v = res.results[core]["out"].reshape(CIN, COUT, R)  # (c, d, r)
        out[core * R : (core + 1) * R, :] = v.sum(axis=0).T
    out = out.reshape(B, L, CIN)
    return _fixup_head(out, times, features, emb, W1, B1, W2, B2, W3, B3, W4, B4)
